# revision 41
# baseline (speedup 1.0000x reference)
"""Block-sparse (banded) attention kernel for Trainium2, 8 NeuronCores.

Sharding: data-parallel over batch (2) x tensor-parallel over heads
(16 heads -> 4 per core).  Each core computes its 4 heads' Q/K/V
projections, banded block attention (|r-c| <= 15 blocks, per-block
softmax), and a partial output projection; the host sums the 4 partial
outputs per batch element.

V2 structure: the band mask is folded into the scores matmul via 32
static contract rows (one-hot q-block indicator on the moving side x
-3e4 band-complement table on the stationary side), so masked scores
exp to exactly 0.  Per-block softmax denominators come from ONE matmul
with a block-membership (+eps) stationary whose output is already
broadcast across partitions; reciprocal runs per pair on the vector
engine.  Each pair only processes its valid contiguous q-range.

Self-contained: hardcodes all shapes; only needs the concourse tree that
the environment already puts on sys.path.
"""

import sys

for _p in ("/opt/trn_rl_repo",):
    if _p not in sys.path:
        sys.path.insert(0, _p)

from contextlib import ExitStack

import numpy as np
import ml_dtypes

import concourse.bacc as bacc
import concourse.tile as tile
from concourse import bass_utils, mybir

F32 = mybir.dt.float32
BF16 = mybir.dt.bfloat16
EXP = mybir.ActivationFunctionType.Exp
BF16NP = ml_dtypes.bfloat16

B, S, E = 2, 2048, 1024
H, HD, BLK = 16, 64, 64
NB = S // BLK  # 32 blocks
NCORES = 8
HPC = 4  # heads per core
F = HPC * HD  # 256 local features
BAND = 15
SCALE = HD ** -0.5
BIGNEG = -30000.0  # masked-score bias; exp underflows to exactly 0 in f32
EPS_BG = 1e-20  # background weight in the sum stationary: keeps denom > 0

# per r8-slab (8 query blocks, q=512) column-block ranges, even-extended
T_SLABS = 4
QS = 512  # q extent per slab
LO = []
NP_T = []
for _t in range(T_SLABS):
    lo = max(0, 8 * _t - BAND)
    hi = min(NB - 1, 8 * _t + 7 + BAND)
    if (hi - lo + 1) % 2 == 1:
        if lo > 0:
            lo -= 1
        else:
            hi += 1
    LO.append(lo)
    NP_T.append((hi - lo + 1) // 2)
MAXP = max(NP_T)  # 16 pairs

# per (slab, pair): valid contiguous local q-block range [lb, ub]
#   union of the two blocks' bands: global r in [c0-15, c0+16]
QRANGE = []  # QRANGE[t][j] = (qlo, qhi) in elements within the slab
PAIR_ORDER = []  # full-width pair first (accumulation-group opener)
for _t in range(T_SLABS):
    rng = []
    for _j in range(NP_T[_t]):
        c0 = LO[_t] + 2 * _j
        lb = max(0, c0 - BAND - 8 * _t)
        ub = min(7, c0 + BAND + 1 - 8 * _t)
        assert lb <= ub
        rng.append((lb * BLK, (ub + 1) * BLK))
    QRANGE.append(rng)
    full = [j for j, (a, b) in enumerate(rng) if b - a == QS]
    order = [full[0]] + [j for j in range(NP_T[_t]) if j != full[0]]
    PAIR_ORDER.append(order)

# compact expS layout: pair (order index i) starts at CUMOFF[t][i]; the
# sum/recip/normalize steps are positional (block-agnostic), so they run on
# full 512-wide chunks of the compact layout.  CHUNKS_AT[t][i] lists chunk
# starts that complete when pair i's exp lands.  Unit widths are 512-aligned.
CUMOFF = []
CHUNKS_AT = []
NCHUNK = []
for _t in range(T_SLABS):
    offs = []
    cum = 0
    chunks_at = {}
    prevc = 0
    for _i, _j in enumerate(PAIR_ORDER[_t]):
        a, b = QRANGE[_t][_j]
        offs.append(cum)
        cum += b - a
        newc = cum // QS
        if newc > prevc:
            chunks_at[_i] = list(range(prevc, newc))
            prevc = newc
    assert cum % QS == 0
    CUMOFF.append(offs)
    CHUNKS_AT.append(chunks_at)
    NCHUNK.append(cum // QS)


def build_nc():
    nc = bacc.Bacc("TRN2", target_bir_lowering=False, debug=False)

    xq_d = nc.dram_tensor("xqT", [E, S], BF16, kind="ExternalInput")
    xk_d = nc.dram_tensor("xkT", [E, S], BF16, kind="ExternalInput")
    xv_d = nc.dram_tensor("xvT", [E, S], BF16, kind="ExternalInput")
    wq_d = nc.dram_tensor("wqT", [E, F], BF16, kind="ExternalInput")
    wk_d = nc.dram_tensor("wkT", [E, F], BF16, kind="ExternalInput")
    wv_d = nc.dram_tensor("wvT", [E, F], BF16, kind="ExternalInput")
    wo_d = nc.dram_tensor("woT", [F, E], BF16, kind="ExternalInput")
    qhot_d = nc.dram_tensor("qhot", [32, S], BF16, kind="ExternalInput")
    kband_d = nc.dram_tensor("kband", [32, S], BF16, kind="ExternalInput")
    sbc_d = nc.dram_tensor("sbc", [128, 128], BF16, kind="ExternalInput")
    out_d = nc.dram_tensor("out", [S, E], BF16, kind="ExternalOutput")

    with tile.TileContext(nc) as tc, ExitStack() as ctx, nc.allow_low_precision(
        reason="bf16 pipeline; fp32 PSUM accumulate throughout"
    ):
        pers = ctx.enter_context(tc.tile_pool(name="pers", bufs=1))
        qT = pers.tile([96, HPC * S], BF16, tag="qT")
        kT = pers.tile([96, HPC * S], BF16, tag="kT")
        vv = pers.tile([128, 16 * F], BF16, tag="vv")
        wq = pers.tile([128, 8 * F], BF16, tag="wq")
        wk = pers.tile([128, 8 * F], BF16, tag="wk")
        wv = pers.tile([128, 8 * F], BF16, tag="wv")
        wo2 = pers.tile([128, 2 * E], BF16, tag="wo2")
        sbc = pers.tile([128, 128], BF16, tag="sbc")
        xvbig = pers.tile([128, 8 * 2048], BF16, tag="xvbig")

        # k-projection weights first: phase 1 is on the critical path
        nc.sync.dma_start(
            wk[:].rearrange("p (c f) -> p c f", c=8),
            wk_d.ap().rearrange("(c p) f -> p c f", p=128),
        )
        # remaining weights/constants via gpsimd (SWDGE) so they don't
        # queue ahead of the phase-1/2 x-tile loads on the sync ring
        nc.gpsimd.dma_start(
            wv[:].rearrange("p (c f) -> p c f", c=8),
            wv_d.ap().rearrange("(c p) f -> p c f", p=128),
        )
        nc.gpsimd.dma_start(
            wq[:].rearrange("p (c f) -> p c f", c=8),
            wq_d.ap().rearrange("(c p) f -> p c f", p=128),
        )
        # wo packed 2 heads deep: partition (h%2)*64+d, free (h//2)*E+e
        nc.gpsimd.dma_start(
            wo2[:].rearrange("p (hh e) -> p hh e", hh=2),
            wo_d.ap().rearrange("(hh two d) e -> (two d) hh e", hh=2, two=2),
        )
        nc.gpsimd.dma_start(sbc[:], sbc_d.ap())
        # static contract rows 64..95 of qT/kT, replicated per head fold
        nc.gpsimd.dma_start(
            qT[64:96, :].rearrange("p (h s) -> p h s", h=HPC),
            qhot_d.ap().rearrange("p s -> p () s").broadcast_to((32, HPC, S)),
        )
        nc.gpsimd.dma_start(
            kT[64:96, :].rearrange("p (h s) -> p h s", h=HPC),
            kband_d.ap().rearrange("p s -> p () s").broadcast_to((32, HPC, S)),
        )

        # ---- phase 1: k projection (kT layout [head, f, s]) ----
        with tc.tile_pool(name="xk", bufs=4) as xkp, tc.tile_pool(
            name="psK", bufs=1, space="PSUM"
        ) as pskp:
            # phase-2 x preload: per-sc grouped loads issued near the end of
            # the xk stream — sc0 lands as phase-1 compute drains, later
            # chunks pipeline behind phase-2's sc-major consumption
            def xv_load(sc):
                nc.sync.dma_start(
                    xvbig[:, sc * 4096 : (sc + 1) * 4096].rearrange(
                        "p (c s) -> p c s", c=8
                    ),
                    xv_d.ap().rearrange("(c p) s -> p c s", p=128)[
                        :, :, sc * 512 : (sc + 1) * 512
                    ],
                )

            psK = pskp.tile([128, 4096], F32)
            for e in range(8):
                xt = xkp.tile([128, S], BF16, tag="xk")
                nc.sync.dma_start(xt[:], xk_d.ap()[e * 128 : (e + 1) * 128, :])
                if e == 7:
                    # after the last xk chunk: keep the xk stream unimpeded
                    for sc in range(4):
                        xv_load(sc)
                for fold in range(2):
                    for sc in range(4):
                        nc.tensor.matmul(
                            psK[:, (fold * 4 + sc) * 512 : (fold * 4 + sc + 1) * 512],
                            wk[:, e * F + fold * 128 : e * F + fold * 128 + 128],
                            xt[:, sc * 512 : (sc + 1) * 512],
                            start=(e == 0),
                            stop=(e == 7),
                        )
            for fold in range(2):
                for sc in range(4):
                    src = psK[:, (fold * 4 + sc) * 512 : (fold * 4 + sc + 1) * 512]
                    h0, h1 = 2 * fold, 2 * fold + 1
                    # split across scalar and DVE: vproj's PSUM reuse waits on
                    # this whole chain, so halve its length
                    nc.scalar.copy(
                        kT[0:64, h0 * S + sc * 512 : h0 * S + (sc + 1) * 512],
                        src[0:64, :],
                    )
                    nc.vector.tensor_copy(
                        kT[0:64, h1 * S + sc * 512 : h1 * S + (sc + 1) * 512],
                        src[64:128, :],
                    )

        # ---- phase 2: v projection (natural layout [s, f]) ----
        with tc.tile_pool(name="psV", bufs=2, space="PSUM") as psvp:
            for sc in range(4):
                pvs = [
                    psvp.tile([128, 256], F32, name=f"pv{sub}", tag=f"psV{sub}")
                    for sub in range(4)
                ]
                for e in range(8):
                    for sub in range(4):
                        nc.tensor.matmul(
                            pvs[sub][:],
                            xvbig[:, sc * 4096 + e * 512 + sub * 128 :
                                  sc * 4096 + e * 512 + (sub + 1) * 128],
                            wv[:, e * F : (e + 1) * F],
                            start=(e == 0),
                            stop=(e == 7),
                        )
                for sub in range(4):
                    # split PSUM->SBUF copies across scalar and DVE so the
                    # bank-recycle chain drains twice as fast
                    dst = vv[:, sc * 1024 + sub * 256 : sc * 1024 + (sub + 1) * 256]
                    if sub < 2:
                        nc.scalar.copy(dst, pvs[sub][:])
                    else:
                        nc.vector.tensor_copy(dst, pvs[sub][:])

        # ---- phase 3: q projection + attention + output projection ----
        xqp = ctx.enter_context(tc.tile_pool(name="xq", bufs=2))
        psSp = ctx.enter_context(tc.tile_pool(name="psS", bufs=6, space="PSUM"))
        flexp = ctx.enter_context(tc.tile_pool(name="flex", bufs=2, space="PSUM"))
        expp = ctx.enter_context(tc.tile_pool(name="expS", bufs=2))
        ptcp = ctx.enter_context(tc.tile_pool(name="ptc", bufs=2))
        rbsp = ctx.enter_context(tc.tile_pool(name="rbs", bufs=4))
        attp = ctx.enter_context(tc.tile_pool(name="att", bufs=4))
        outp = ctx.enter_context(tc.tile_pool(name="outsb", bufs=2))

        xq_tiles = {}

        def qproj_load(t):
            # one grouped load for the whole slab's x columns
            xt = xqp.tile([128, 8 * 512], BF16, tag="xq")
            nc.sync.dma_start(
                xt[:].rearrange("p (c s) -> p c s", c=8),
                xq_d.ap().rearrange("(c p) s -> p c s", p=128)[
                    :, :, t * 512 : (t + 1) * 512
                ],
            )
            xq_tiles[t] = xt

        def qproj_mm(t):
            xt = xq_tiles.pop(t)
            pqs = [
                flexp.tile([128, 512], F32, name=f"pq{fold}", tag="flex")
                for fold in range(2)
            ]
            for e in range(8):
                for fold in range(2):
                    nc.tensor.matmul(
                        pqs[fold][:],
                        wq[:, e * F + fold * 128 : e * F + fold * 128 + 128],
                        xt[:, e * 512 : (e + 1) * 512],
                        start=(e == 0),
                        stop=(e == 7),
                    )
            for fold in range(2):
                h0, h1 = 2 * fold, 2 * fold + 1
                nc.scalar.copy(
                    qT[0:64, h0 * S + t * QS : h0 * S + (t + 1) * QS],
                    pqs[fold][0:64, :],
                )
                nc.scalar.copy(
                    qT[0:64, h1 * S + t * QS : h1 * S + (t + 1) * QS],
                    pqs[fold][64:128, :],
                )

        def outproj(t, att2s):
            for sc2 in range(4):
                ob = outp.tile([128, 1024], BF16, tag="outsb")
                for eh in range(2):
                    po = flexp.tile([128, 512], F32, tag="flex")
                    for hh in range(2):
                        nc.tensor.matmul(
                            po[:],
                            att2s[hh][:, sc2 * 128 : sc2 * 128 + 128],
                            wo2[:, hh * E + eh * 512 : hh * E + eh * 512 + 512],
                            start=(hh == 0),
                            stop=(hh == 1),
                        )
                    nc.scalar.copy(ob[:, eh * 512 : (eh + 1) * 512], po[:])
                row = (4 * t + sc2) * 128
                nc.gpsimd.dma_start(out_d.ap()[row : row + 128, :], ob[:])

        # flat software pipeline over every (slab, head, pair) slot
        SLOTS = [
            (t, h, i)
            for t in range(T_SLABS)
            for h in range(HPC)
            for i in range(NP_T[t])
        ]
        N = len(SLOTS)
        ctx = {}
        atts_by_t = {t: [] for t in range(T_SLABS)}

        att2_by = {}

        def ensure_ctx(t, h):
            if (t, h) in ctx:
                return
            if h == 0 and t == 0:
                qproj_mm(0)
            if h % 2 == 0:
                att2_by[(t, h // 2)] = attp.tile(
                    [128, 512], BF16, name="att2", tag="att"
                )
            ctx[(t, h)] = {
                "expS": expp.tile([128, MAXP * QS], BF16, name="expS", tag="expS"),
                "ptc": ptcp.tile([128, MAXP * QS], BF16, name="ptc", tag="ptc"),
                "acco": psSp.tile([128, 512], F32, name="acco", tag="psS"),
                "attn": att2_by[(t, h // 2)],
            }
            if t + 1 < T_SLABS:
                if h == HPC - 2:
                    # start next slab's x transfer one unit before its matmuls
                    qproj_load(t + 1)
                elif h == HPC - 1:
                    qproj_mm(t + 1)

        def do_scores(g):
            t, h, i = SLOTS[g]
            ensure_ctx(t, h)
            c = ctx[(t, h)]
            j = PAIR_ORDER[t][i]
            c0 = LO[t] + 2 * j
            qlo, qhi = QRANGE[t][j]
            w = qhi - qlo
            ps = psSp.tile([128, 512], F32, name="ps", tag="psS")
            nc.tensor.matmul(
                ps[:, 0:w],
                kT[0:96, h * S + c0 * 64 : h * S + c0 * 64 + 128],
                qT[0:96, h * S + t * QS + qlo : h * S + t * QS + qhi],
                start=True,
                stop=True,
            )
            off = CUMOFF[t][i]
            nc.scalar.activation(c["expS"][:, off : off + w], ps[:, 0:w], EXP)

        def do_sums(g):
            t, h, i = SLOTS[g]
            c = ctx[(t, h)]
            # emit full 512-wide chunks of the compact layout that pair i's
            # exp just completed — sum/recip/normalize are block-agnostic
            for cidx in CHUNKS_AT[t].get(i, []):
                c0 = cidx * QS
                bs = psSp.tile([128, 512], F32, name="bs", tag="psS")
                nc.tensor.matmul(
                    bs[:],
                    sbc[:, :],
                    c["expS"][:, c0 : c0 + QS],
                    start=True,
                    stop=True,
                )
                rbs = rbsp.tile([128, 512], F32, tag="rbs")
                nc.vector.reciprocal_approx_fast(rbs[:], bs[:])
                # alternate the normalize multiply between DVE and the idle
                # Pool engine; keep unit-tail chunks on the faster DVE
                eng = (
                    nc.gpsimd
                    if (cidx % 2 == 1 and cidx < NCHUNK[t] - 2)
                    else nc.vector
                )
                eng.tensor_mul(
                    c["ptc"][:, c0 : c0 + QS], c["expS"][:, c0 : c0 + QS], rbs[:]
                )

        def do_av(g):
            t, h, i = SLOTS[g]
            c = ctx[(t, h)]
            j = PAIR_ORDER[t][i]
            c0 = LO[t] + 2 * j
            qlo, qhi = QRANGE[t][j]
            w = qhi - qlo
            cp = c0 // 2
            off = CUMOFF[t][i]
            rb = 64 * (h % 2)  # odd heads land on PSUM partitions 64-127 so
            # the attn copy into the packed att2 tile stays partition-aligned
            nc.tensor.matmul(
                c["acco"][rb : rb + 64, qlo:qhi],
                vv[:, cp * F + h * 64 : cp * F + h * 64 + 64],
                c["ptc"][:, off : off + w],
                start=(i == 0),
                stop=(i == NP_T[t] - 1),
                skip_group_check=True,
            )
            if i == NP_T[t] - 1:
                nc.scalar.copy(
                    c["attn"][rb : rb + 64, :], c["acco"][rb : rb + 64, :]
                )
                if h % 2 == 1:
                    atts_by_t[t].append(c["attn"])
                    if len(atts_by_t[t]) == 2:
                        outproj(t, atts_by_t[t])

        qproj_load(0)  # transfer overlaps phase-2 compute
        do_scores(0)
        do_scores(1)
        for g in range(N):
            do_sums(g)
            if g + 2 < N:
                do_scores(g + 2)
            if g >= 2:
                do_av(g - 2)
        do_av(N - 2)
        do_av(N - 1)

    nc.compile()
    return nc


_NC_CACHE = []


def _get_nc():
    if not _NC_CACHE:
        _NC_CACHE.append(build_nc())
    return _NC_CACHE[0]


def _host_consts():
    qhot = np.zeros((32, S), np.float32)
    for s in range(S):
        qhot[s // BLK, s] = 1.0
    kband = np.zeros((32, S), np.float32)
    for k in range(S):
        c = k // BLK
        for r in range(32):
            if abs(r - c) > BAND:
                kband[r, k] = BIGNEG
    sbc = np.full((128, 128), EPS_BG, np.float32)
    for k in range(128):
        for p in range(128):
            if k // 64 == p // 64:
                sbc[k, p] = 1.0
    return qhot, kband, sbc


def make_in_maps(query, key, value, Wq, Wk, Wv, Wo):
    query = np.asarray(query, np.float32)
    key = np.asarray(key, np.float32)
    value = np.asarray(value, np.float32)
    Wq = np.asarray(Wq, np.float32)
    Wk = np.asarray(Wk, np.float32)
    Wv = np.asarray(Wv, np.float32)
    Wo = np.asarray(Wo, np.float32)

    qhot, kband, sbc = _host_consts()

    in_maps = []
    for c in range(NCORES):
        b, g = divmod(c, HPC)
        fs = slice(F * g, F * (g + 1))
        in_maps.append(
            {
                "xqT": np.ascontiguousarray(query[b].T).astype(BF16NP),
                "xkT": np.ascontiguousarray(key[b].T).astype(BF16NP),
                "xvT": np.ascontiguousarray(value[b].T).astype(BF16NP),
                "wqT": np.ascontiguousarray((Wq[fs, :] * SCALE).T).astype(BF16NP),
                "wkT": np.ascontiguousarray(Wk[fs, :].T).astype(BF16NP),
                "wvT": np.ascontiguousarray(Wv[fs, :].T).astype(BF16NP),
                "woT": np.ascontiguousarray(Wo[:, fs].T).astype(BF16NP),
                "qhot": qhot.astype(BF16NP),
                "kband": kband.astype(BF16NP),
                "sbc": sbc.astype(BF16NP),
            }
        )
    return in_maps


def kernel(query, key, value, Wq, Wk, Wv, Wo):
    nc = _get_nc()
    in_maps = make_in_maps(query, key, value, Wq, Wk, Wv, Wo)
    res = bass_utils.run_bass_kernel_spmd(nc, in_maps, core_ids=list(range(NCORES)))
    out = np.zeros((B, S, E), np.float32)
    for c in range(NCORES):
        b = c // HPC
        out[b] += res.results[c]["out"]
    return out


# revision 44
# speedup vs baseline: 1.1493x; 1.1493x over previous
"""Block-sparse (banded) attention kernel for Trainium2, 8 NeuronCores.

Sharding: data-parallel over batch (2) x tensor-parallel over heads
(16 heads -> 4 per core).  Each core computes its 4 heads' Q/K/V
projections, banded block attention (|r-c| <= 15 blocks, per-block
softmax), and a partial output projection; the host sums the 4 partial
outputs per batch element.

V2 structure: the band mask is folded into the scores matmul via 32
static contract rows (one-hot q-block indicator on the moving side x
-3e4 band-complement table on the stationary side), so masked scores
exp to exactly 0.  Per-block softmax denominators come from ONE matmul
with a block-membership (+eps) stationary whose output is already
broadcast across partitions; reciprocal runs per pair on the vector
engine.  Each pair only processes its valid contiguous q-range.

Self-contained: hardcodes all shapes; only needs the concourse tree that
the environment already puts on sys.path.
"""

import sys

for _p in ("/opt/trn_rl_repo",):
    if _p not in sys.path:
        sys.path.insert(0, _p)

from contextlib import ExitStack

import numpy as np
import ml_dtypes

import concourse.bacc as bacc
import concourse.tile as tile
from concourse import bass_utils, mybir

F32 = mybir.dt.float32
BF16 = mybir.dt.bfloat16
EXP = mybir.ActivationFunctionType.Exp
BF16NP = ml_dtypes.bfloat16

B, S, E = 2, 2048, 1024
H, HD, BLK = 16, 64, 64
NB = S // BLK  # 32 blocks
NCORES = 8
HPC = 4  # heads per core
F = HPC * HD  # 256 local features
BAND = 15
SCALE = HD ** -0.5
BIGNEG = -30000.0  # masked-score bias; exp underflows to exactly 0 in f32
EPS_BG = 1e-20  # background weight in the sum stationary: keeps denom > 0

# per r8-slab (8 query blocks, q=512) column-block ranges, even-extended
T_SLABS = 4
QS = 512  # q extent per slab
LO = []
NP_T = []
for _t in range(T_SLABS):
    lo = max(0, 8 * _t - BAND)
    hi = min(NB - 1, 8 * _t + 7 + BAND)
    if (hi - lo + 1) % 2 == 1:
        if lo > 0:
            lo -= 1
        else:
            hi += 1
    LO.append(lo)
    NP_T.append((hi - lo + 1) // 2)
MAXP = max(NP_T)  # 16 pairs

# per (slab, pair): valid contiguous local q-block range [lb, ub]
#   union of the two blocks' bands: global r in [c0-15, c0+16]
QRANGE = []  # QRANGE[t][j] = (qlo, qhi) in elements within the slab
PAIR_ORDER = []  # full-width pair first (accumulation-group opener)
for _t in range(T_SLABS):
    rng = []
    for _j in range(NP_T[_t]):
        c0 = LO[_t] + 2 * _j
        lb = max(0, c0 - BAND - 8 * _t)
        ub = min(7, c0 + BAND + 1 - 8 * _t)
        assert lb <= ub
        rng.append((lb * BLK, (ub + 1) * BLK))
    QRANGE.append(rng)
    full = [j for j, (a, b) in enumerate(rng) if b - a == QS]
    part = [j for j, (a, b) in enumerate(rng) if b - a < QS]
    # full-width pair opens the accumulation group; partial pairs go EARLY so
    # normalize chunks never complete late at the unit tail (clean drains)
    order = [full[0]] + part + [j for j in full[1:]]
    PAIR_ORDER.append(order)

# compact expS layout: pair (order index i) starts at CUMOFF[t][i]; the
# sum/recip/normalize steps are positional (block-agnostic), so they run on
# full 512-wide chunks of the compact layout.  CHUNKS_AT[t][i] lists chunk
# indices that complete when pair i's exp lands.  Unit widths are 512-aligned.
CUMOFF = []
CHUNKS_AT = []
NCHUNK = []
for _t in range(T_SLABS):
    offs = []
    cum = 0
    chunks_at = {}
    prevc = 0
    for _i, _j in enumerate(PAIR_ORDER[_t]):
        a, b = QRANGE[_t][_j]
        offs.append(cum)
        cum += b - a
        newc = cum // QS
        if newc > prevc:
            chunks_at[_i] = list(range(prevc, newc))
            prevc = newc
    assert cum % QS == 0
    CUMOFF.append(offs)
    CHUNKS_AT.append(chunks_at)
    NCHUNK.append(cum // QS)


def build_nc():
    nc = bacc.Bacc("TRN2", target_bir_lowering=False, debug=False)

    xq_d = nc.dram_tensor("xqT", [E, S], BF16, kind="ExternalInput")
    xk_d = nc.dram_tensor("xkT", [E, S], BF16, kind="ExternalInput")
    xv_d = nc.dram_tensor("xvT", [E, S], BF16, kind="ExternalInput")
    wq_d = nc.dram_tensor("wqT", [E, F], BF16, kind="ExternalInput")
    wk_d = nc.dram_tensor("wkT", [E, F], BF16, kind="ExternalInput")
    wv_d = nc.dram_tensor("wvT", [E, F], BF16, kind="ExternalInput")
    wo_d = nc.dram_tensor("woT", [F, E], BF16, kind="ExternalInput")
    qhot_d = nc.dram_tensor("qhot", [32, S], BF16, kind="ExternalInput")
    kband_d = nc.dram_tensor("kband", [32, S], BF16, kind="ExternalInput")
    sbc_d = nc.dram_tensor("sbc", [128, 128], BF16, kind="ExternalInput")
    out_d = nc.dram_tensor("out", [S, E], BF16, kind="ExternalOutput")

    with tile.TileContext(nc) as tc, ExitStack() as ctx, nc.allow_low_precision(
        reason="bf16 pipeline; fp32 PSUM accumulate throughout"
    ):
        pers = ctx.enter_context(tc.tile_pool(name="pers", bufs=1))
        qT = pers.tile([96, HPC * S], BF16, tag="qT")
        kT = pers.tile([96, HPC * S], BF16, tag="kT")
        vv = pers.tile([128, 16 * F], BF16, tag="vv")
        wq = pers.tile([128, 8 * F], BF16, tag="wq")
        wk = pers.tile([128, 8 * F], BF16, tag="wk")
        wv = pers.tile([128, 8 * F], BF16, tag="wv")
        wo2 = pers.tile([128, 2 * E], BF16, tag="wo2")
        sbc = pers.tile([128, 128], BF16, tag="sbc")
        xvbig = pers.tile([128, 8 * 2048], BF16, tag="xvbig")

        # k-projection weights first: phase 1 is on the critical path
        nc.sync.dma_start(
            wk[:].rearrange("p (c f) -> p c f", c=8),
            wk_d.ap().rearrange("(c p) f -> p c f", p=128),
        )
        # remaining weights/constants via gpsimd (SWDGE) so they don't
        # queue ahead of the phase-1/2 x-tile loads on the sync ring
        nc.gpsimd.dma_start(
            wv[:].rearrange("p (c f) -> p c f", c=8),
            wv_d.ap().rearrange("(c p) f -> p c f", p=128),
        )
        nc.gpsimd.dma_start(
            wq[:].rearrange("p (c f) -> p c f", c=8),
            wq_d.ap().rearrange("(c p) f -> p c f", p=128),
        )
        # wo packed 2 heads deep: partition (h%2)*64+d, free (h//2)*E+e
        nc.gpsimd.dma_start(
            wo2[:].rearrange("p (hh e) -> p hh e", hh=2),
            wo_d.ap().rearrange("(hh two d) e -> (two d) hh e", hh=2, two=2),
        )
        nc.gpsimd.dma_start(sbc[:], sbc_d.ap())
        # static contract rows 64..95 of qT/kT, replicated per head fold
        nc.gpsimd.dma_start(
            qT[64:96, :].rearrange("p (h s) -> p h s", h=HPC),
            qhot_d.ap().rearrange("p s -> p () s").broadcast_to((32, HPC, S)),
        )
        nc.gpsimd.dma_start(
            kT[64:96, :].rearrange("p (h s) -> p h s", h=HPC),
            kband_d.ap().rearrange("p s -> p () s").broadcast_to((32, HPC, S)),
        )

        # ---- phase 1: k projection (kT layout [head, f, s]) ----
        with tc.tile_pool(name="xk", bufs=4) as xkp, tc.tile_pool(
            name="psK", bufs=1, space="PSUM"
        ) as pskp:
            # phase-2 x preload: per-sc grouped loads issued near the end of
            # the xk stream — sc0 lands as phase-1 compute drains, later
            # chunks pipeline behind phase-2's sc-major consumption
            def xv_load(sc):
                nc.sync.dma_start(
                    xvbig[:, sc * 4096 : (sc + 1) * 4096].rearrange(
                        "p (c s) -> p c s", c=8
                    ),
                    xv_d.ap().rearrange("(c p) s -> p c s", p=128)[
                        :, :, sc * 512 : (sc + 1) * 512
                    ],
                )

            psK = pskp.tile([128, 4096], F32)
            for e in range(8):
                xt = xkp.tile([128, S], BF16, tag="xk")
                nc.sync.dma_start(xt[:], xk_d.ap()[e * 128 : (e + 1) * 128, :])
                if e == 7:
                    # after the last xk chunk: keep the xk stream unimpeded
                    for sc in range(4):
                        xv_load(sc)
                for fold in range(2):
                    for sc in range(4):
                        nc.tensor.matmul(
                            psK[:, (fold * 4 + sc) * 512 : (fold * 4 + sc + 1) * 512],
                            wk[:, e * F + fold * 128 : e * F + fold * 128 + 128],
                            xt[:, sc * 512 : (sc + 1) * 512],
                            start=(e == 0),
                            stop=(e == 7),
                        )
            for fold in range(2):
                for sc in range(4):
                    src = psK[:, (fold * 4 + sc) * 512 : (fold * 4 + sc + 1) * 512]
                    h0, h1 = 2 * fold, 2 * fold + 1
                    # split across scalar and DVE: vproj's PSUM reuse waits on
                    # this whole chain, so halve its length
                    nc.scalar.copy(
                        kT[0:64, h0 * S + sc * 512 : h0 * S + (sc + 1) * 512],
                        src[0:64, :],
                    )
                    nc.vector.tensor_copy(
                        kT[0:64, h1 * S + sc * 512 : h1 * S + (sc + 1) * 512],
                        src[64:128, :],
                    )

        # ---- phase 2: v projection (natural layout [s, f]) ----
        with tc.tile_pool(name="psV", bufs=2, space="PSUM") as psvp:
            for sc in range(4):
                pvs = [
                    psvp.tile([128, 256], F32, name=f"pv{sub}", tag=f"psV{sub}")
                    for sub in range(4)
                ]
                for e in range(8):
                    for sub in range(4):
                        nc.tensor.matmul(
                            pvs[sub][:],
                            xvbig[:, sc * 4096 + e * 512 + sub * 128 :
                                  sc * 4096 + e * 512 + (sub + 1) * 128],
                            wv[:, e * F : (e + 1) * F],
                            start=(e == 0),
                            stop=(e == 7),
                        )
                for sub in range(4):
                    # split PSUM->SBUF copies across scalar and DVE so the
                    # bank-recycle chain drains twice as fast
                    dst = vv[:, sc * 1024 + sub * 256 : sc * 1024 + (sub + 1) * 256]
                    if sub < 2:
                        nc.scalar.copy(dst, pvs[sub][:])
                    else:
                        nc.vector.tensor_copy(dst, pvs[sub][:])

        # ---- phase 3: q projection + attention + output projection ----
        xqp = ctx.enter_context(tc.tile_pool(name="xq", bufs=2))
        psSp = ctx.enter_context(tc.tile_pool(name="psS", bufs=6, space="PSUM"))
        flexp = ctx.enter_context(tc.tile_pool(name="flex", bufs=2, space="PSUM"))
        expp = ctx.enter_context(tc.tile_pool(name="expS", bufs=2))
        ptcp = ctx.enter_context(tc.tile_pool(name="ptc", bufs=2))
        rbsp = ctx.enter_context(tc.tile_pool(name="rbs", bufs=4))
        attp = ctx.enter_context(tc.tile_pool(name="att", bufs=4))
        outp = ctx.enter_context(tc.tile_pool(name="outsb", bufs=2))

        xq_tiles = {}

        def qproj_load(t):
            # one grouped load for the whole slab's x columns
            xt = xqp.tile([128, 8 * 512], BF16, tag="xq")
            nc.sync.dma_start(
                xt[:].rearrange("p (c s) -> p c s", c=8),
                xq_d.ap().rearrange("(c p) s -> p c s", p=128)[
                    :, :, t * 512 : (t + 1) * 512
                ],
            )
            xq_tiles[t] = xt

        def qproj_mm(t):
            xt = xq_tiles.pop(t)
            pqs = [
                flexp.tile([128, 512], F32, name=f"pq{fold}", tag="flex")
                for fold in range(2)
            ]
            for e in range(8):
                for fold in range(2):
                    nc.tensor.matmul(
                        pqs[fold][:],
                        wq[:, e * F + fold * 128 : e * F + fold * 128 + 128],
                        xt[:, e * 512 : (e + 1) * 512],
                        start=(e == 0),
                        stop=(e == 7),
                    )
            for fold in range(2):
                h0, h1 = 2 * fold, 2 * fold + 1
                nc.scalar.copy(
                    qT[0:64, h0 * S + t * QS : h0 * S + (t + 1) * QS],
                    pqs[fold][0:64, :],
                )
                nc.scalar.copy(
                    qT[0:64, h1 * S + t * QS : h1 * S + (t + 1) * QS],
                    pqs[fold][64:128, :],
                )

        def outproj(t, att2s):
            for sc2 in range(4):
                ob = outp.tile([128, 1024], BF16, tag="outsb")
                for eh in range(2):
                    po = flexp.tile([128, 512], F32, tag="flex")
                    for hh in range(2):
                        nc.tensor.matmul(
                            po[:],
                            att2s[hh][:, sc2 * 128 : sc2 * 128 + 128],
                            wo2[:, hh * E + eh * 512 : hh * E + eh * 512 + 512],
                            start=(hh == 0),
                            stop=(hh == 1),
                        )
                    nc.scalar.copy(ob[:, eh * 512 : (eh + 1) * 512], po[:])
                row = (4 * t + sc2) * 128
                nc.gpsimd.dma_start(out_d.ap()[row : row + 128, :], ob[:])

        # flat software pipeline over every (slab, head, pair) slot
        SLOTS = [
            (t, h, i)
            for t in range(T_SLABS)
            for h in range(HPC)
            for i in range(NP_T[t])
        ]
        N = len(SLOTS)
        ctx = {}
        atts_by_t = {t: [] for t in range(T_SLABS)}

        att2_by = {}

        def ensure_ctx(t, h):
            if (t, h) in ctx:
                return
            if h == 0 and t == 0:
                qproj_mm(0)
            if h % 2 == 0:
                att2_by[(t, h // 2)] = attp.tile(
                    [128, 512], BF16, name="att2", tag="att"
                )
            ctx[(t, h)] = {
                "expS": expp.tile([128, MAXP * QS], BF16, name="expS", tag="expS"),
                "ptc": ptcp.tile([128, MAXP * QS], BF16, name="ptc", tag="ptc"),
                "acco": psSp.tile([128, 512], F32, name="acco", tag="psS"),
                "attn": att2_by[(t, h // 2)],
            }
            if t + 1 < T_SLABS:
                if h == HPC - 2:
                    # start next slab's x transfer one unit before its matmuls
                    qproj_load(t + 1)
                elif h == HPC - 1:
                    qproj_mm(t + 1)

        def do_scores(g):
            t, h, i = SLOTS[g]
            ensure_ctx(t, h)
            c = ctx[(t, h)]
            j = PAIR_ORDER[t][i]
            c0 = LO[t] + 2 * j
            qlo, qhi = QRANGE[t][j]
            w = qhi - qlo
            ps = psSp.tile([128, 512], F32, name="ps", tag="psS")
            nc.tensor.matmul(
                ps[:, 0:w],
                kT[0:96, h * S + c0 * 64 : h * S + c0 * 64 + 128],
                qT[0:96, h * S + t * QS + qlo : h * S + t * QS + qhi],
                start=True,
                stop=True,
            )
            off = CUMOFF[t][i]
            nc.scalar.activation(c["expS"][:, off : off + w], ps[:, 0:w], EXP)

        def do_sums(g):
            t, h, i = SLOTS[g]
            c = ctx[(t, h)]
            # emit the full 512-wide chunks of the compact layout that pair
            # i's exp completed — sum/recip/normalize are block-agnostic
            for cidx in CHUNKS_AT[t].get(i, []):
                c0c = cidx * QS
                bs = psSp.tile([128, 512], F32, name="bs", tag="psS")
                nc.tensor.matmul(
                    bs[:],
                    sbc[:, :],
                    c["expS"][:, c0c : c0c + QS],
                    start=True,
                    stop=True,
                )
                rbs = rbsp.tile([128, 512], F32, tag="rbs")
                nc.vector.reciprocal_approx_fast(rbs[:], bs[:])
                # alternate the normalize multiply between DVE and the idle
                # Pool engine; keep unit-tail chunks on the faster DVE
                eng = (
                    nc.gpsimd
                    if (cidx % 2 == 1 and cidx < NCHUNK[t] - 2)
                    else nc.vector
                )
                eng.tensor_mul(
                    c["ptc"][:, c0c : c0c + QS], c["expS"][:, c0c : c0c + QS],
                    rbs[:],
                )

        def do_av(g):
            t, h, i = SLOTS[g]
            c = ctx[(t, h)]
            j = PAIR_ORDER[t][i]
            c0 = LO[t] + 2 * j
            qlo, qhi = QRANGE[t][j]
            w = qhi - qlo
            cp = c0 // 2
            off = CUMOFF[t][i]
            rb = 64 * (h % 2)  # odd heads land on PSUM partitions 64-127 so
            # the attn copy into the packed att2 tile stays partition-aligned
            nc.tensor.matmul(
                c["acco"][rb : rb + 64, qlo:qhi],
                vv[:, cp * F + h * 64 : cp * F + h * 64 + 64],
                c["ptc"][:, off : off + w],
                start=(i == 0),
                stop=(i == NP_T[t] - 1),
                skip_group_check=True,
            )
            if i == NP_T[t] - 1:
                nc.scalar.copy(
                    c["attn"][rb : rb + 64, :], c["acco"][rb : rb + 64, :]
                )
                if h % 2 == 1:
                    atts_by_t[t].append(c["attn"])
                    if len(atts_by_t[t]) == 2:
                        outproj(t, atts_by_t[t])

        qproj_load(0)  # transfer overlaps phase-2 compute
        do_scores(0)
        do_scores(1)
        for g in range(N):
            do_sums(g)
            if g + 2 < N:
                do_scores(g + 2)
            if g >= 2:
                do_av(g - 2)
        do_av(N - 2)
        do_av(N - 1)

    nc.compile()
    return nc


_NC_CACHE = []


def _get_nc():
    if not _NC_CACHE:
        _NC_CACHE.append(build_nc())
    return _NC_CACHE[0]


def _host_consts():
    qhot = np.zeros((32, S), np.float32)
    for s in range(S):
        qhot[s // BLK, s] = 1.0
    kband = np.zeros((32, S), np.float32)
    for k in range(S):
        c = k // BLK
        for r in range(32):
            if abs(r - c) > BAND:
                kband[r, k] = BIGNEG
    sbc = np.full((128, 128), EPS_BG, np.float32)
    for k in range(128):
        for p in range(128):
            if k // 64 == p // 64:
                sbc[k, p] = 1.0
    return qhot, kband, sbc


def make_in_maps(query, key, value, Wq, Wk, Wv, Wo):
    query = np.asarray(query, np.float32)
    key = np.asarray(key, np.float32)
    value = np.asarray(value, np.float32)
    Wq = np.asarray(Wq, np.float32)
    Wk = np.asarray(Wk, np.float32)
    Wv = np.asarray(Wv, np.float32)
    Wo = np.asarray(Wo, np.float32)

    qhot, kband, sbc = _host_consts()

    in_maps = []
    for c in range(NCORES):
        b, g = divmod(c, HPC)
        fs = slice(F * g, F * (g + 1))
        in_maps.append(
            {
                "xqT": np.ascontiguousarray(query[b].T).astype(BF16NP),
                "xkT": np.ascontiguousarray(key[b].T).astype(BF16NP),
                "xvT": np.ascontiguousarray(value[b].T).astype(BF16NP),
                "wqT": np.ascontiguousarray((Wq[fs, :] * SCALE).T).astype(BF16NP),
                "wkT": np.ascontiguousarray(Wk[fs, :].T).astype(BF16NP),
                "wvT": np.ascontiguousarray(Wv[fs, :].T).astype(BF16NP),
                "woT": np.ascontiguousarray(Wo[:, fs].T).astype(BF16NP),
                "qhot": qhot.astype(BF16NP),
                "kband": kband.astype(BF16NP),
                "sbc": sbc.astype(BF16NP),
            }
        )
    return in_maps


def kernel(query, key, value, Wq, Wk, Wv, Wo):
    nc = _get_nc()
    in_maps = make_in_maps(query, key, value, Wq, Wk, Wv, Wo)
    res = bass_utils.run_bass_kernel_spmd(nc, in_maps, core_ids=list(range(NCORES)))
    out = np.zeros((B, S, E), np.float32)
    for c in range(NCORES):
        b = c // HPC
        out[b] += res.results[c]["out"]
    return out


# revision 45
# speedup vs baseline: 1.1661x; 1.0146x over previous
"""Block-sparse (banded) attention kernel for Trainium2, 8 NeuronCores.

Sharding: data-parallel over batch (2) x tensor-parallel over heads
(16 heads -> 4 per core).  Each core computes its 4 heads' Q/K/V
projections, banded block attention (|r-c| <= 15 blocks, per-block
softmax), and a partial output projection; the host sums the 4 partial
outputs per batch element.

V2 structure: the band mask is folded into the scores matmul via 32
static contract rows (one-hot q-block indicator on the moving side x
-3e4 band-complement table on the stationary side), so masked scores
exp to exactly 0.  Per-block softmax denominators come from ONE matmul
with a block-membership (+eps) stationary whose output is already
broadcast across partitions; reciprocal runs per pair on the vector
engine.  Each pair only processes its valid contiguous q-range.

Self-contained: hardcodes all shapes; only needs the concourse tree that
the environment already puts on sys.path.
"""

import sys

for _p in ("/opt/trn_rl_repo",):
    if _p not in sys.path:
        sys.path.insert(0, _p)

from contextlib import ExitStack

import numpy as np
import ml_dtypes

import concourse.bacc as bacc
import concourse.tile as tile
from concourse import bass_utils, mybir

F32 = mybir.dt.float32
BF16 = mybir.dt.bfloat16
EXP = mybir.ActivationFunctionType.Exp
BF16NP = ml_dtypes.bfloat16

B, S, E = 2, 2048, 1024
H, HD, BLK = 16, 64, 64
NB = S // BLK  # 32 blocks
NCORES = 8
HPC = 4  # heads per core
F = HPC * HD  # 256 local features
BAND = 15
SCALE = HD ** -0.5
BIGNEG = -30000.0  # masked-score bias; exp underflows to exactly 0 in f32
EPS_BG = 1e-20  # background weight in the sum stationary: keeps denom > 0

# per r8-slab (8 query blocks, q=512) column-block ranges, even-extended
T_SLABS = 4
QS = 512  # q extent per slab
LO = []
NP_T = []
for _t in range(T_SLABS):
    lo = max(0, 8 * _t - BAND)
    hi = min(NB - 1, 8 * _t + 7 + BAND)
    if (hi - lo + 1) % 2 == 1:
        if lo > 0:
            lo -= 1
        else:
            hi += 1
    LO.append(lo)
    NP_T.append((hi - lo + 1) // 2)
MAXP = max(NP_T)  # 16 pairs

# per (slab, pair): valid contiguous local q-block range [lb, ub]
#   union of the two blocks' bands: global r in [c0-15, c0+16]
QRANGE = []  # QRANGE[t][j] = (qlo, qhi) in elements within the slab
PAIR_ORDER = []  # full-width pair first (accumulation-group opener)
for _t in range(T_SLABS):
    rng = []
    for _j in range(NP_T[_t]):
        c0 = LO[_t] + 2 * _j
        lb = max(0, c0 - BAND - 8 * _t)
        ub = min(7, c0 + BAND + 1 - 8 * _t)
        assert lb <= ub
        rng.append((lb * BLK, (ub + 1) * BLK))
    QRANGE.append(rng)
    full = [j for j, (a, b) in enumerate(rng) if b - a == QS]
    part = [j for j, (a, b) in enumerate(rng) if b - a < QS]
    # full-width pair opens the accumulation group; partial pairs go EARLY so
    # normalize chunks never complete late at the unit tail (clean drains)
    order = [full[0]] + part + [j for j in full[1:]]
    PAIR_ORDER.append(order)

# compact expS layout: pair (order index i) starts at CUMOFF[t][i]; the
# sum/recip/normalize steps are positional (block-agnostic), so they run on
# full 512-wide chunks of the compact layout.  CHUNKS_AT[t][i] lists chunk
# indices that complete when pair i's exp lands.  Unit widths are 512-aligned.
CUMOFF = []
CHUNKS_AT = []
NCHUNK = []
for _t in range(T_SLABS):
    offs = []
    cum = 0
    chunks_at = {}
    prevc = 0
    for _i, _j in enumerate(PAIR_ORDER[_t]):
        a, b = QRANGE[_t][_j]
        offs.append(cum)
        cum += b - a
        newc = cum // QS
        if newc > prevc:
            chunks_at[_i] = list(range(prevc, newc))
            prevc = newc
    assert cum % QS == 0
    CUMOFF.append(offs)
    CHUNKS_AT.append(chunks_at)
    NCHUNK.append(cum // QS)


def build_nc():
    nc = bacc.Bacc("TRN2", target_bir_lowering=False, debug=False)

    xq_d = nc.dram_tensor("xqT", [E, S], BF16, kind="ExternalInput")
    xk_d = nc.dram_tensor("xkT", [E, S], BF16, kind="ExternalInput")
    xv_d = nc.dram_tensor("xvT", [E, S], BF16, kind="ExternalInput")
    wq_d = nc.dram_tensor("wqT", [E, F], BF16, kind="ExternalInput")
    wk_d = nc.dram_tensor("wkT", [E, F], BF16, kind="ExternalInput")
    wv_d = nc.dram_tensor("wvT", [E, F], BF16, kind="ExternalInput")
    wo_d = nc.dram_tensor("woT", [F, E], BF16, kind="ExternalInput")
    qhot_d = nc.dram_tensor("qhot", [32, S], BF16, kind="ExternalInput")
    kband_d = nc.dram_tensor("kband", [32, S], BF16, kind="ExternalInput")
    sbc_d = nc.dram_tensor("sbc", [128, 128], BF16, kind="ExternalInput")
    out_d = nc.dram_tensor("out", [S, E], BF16, kind="ExternalOutput")

    with tile.TileContext(nc) as tc, ExitStack() as ctx, nc.allow_low_precision(
        reason="bf16 pipeline; fp32 PSUM accumulate throughout"
    ):
        pers = ctx.enter_context(tc.tile_pool(name="pers", bufs=1))
        qT = pers.tile([96, HPC * S], BF16, tag="qT")
        kT = pers.tile([96, HPC * S], BF16, tag="kT")
        vv = pers.tile([128, 16 * F], BF16, tag="vv")
        wq = pers.tile([128, 8 * F], BF16, tag="wq")
        wk = pers.tile([128, 8 * F], BF16, tag="wk")
        wv = pers.tile([128, 8 * F], BF16, tag="wv")
        wo2 = pers.tile([128, 2 * E], BF16, tag="wo2")
        sbc = pers.tile([128, 128], BF16, tag="sbc")
        xvbig = pers.tile([128, 8 * 2048], BF16, tag="xvbig")

        # k-projection weights first: phase 1 is on the critical path
        nc.sync.dma_start(
            wk[:].rearrange("p (c f) -> p c f", c=8),
            wk_d.ap().rearrange("(c p) f -> p c f", p=128),
        )
        # remaining weights/constants via gpsimd (SWDGE) so they don't
        # queue ahead of the phase-1/2 x-tile loads on the sync ring
        nc.gpsimd.dma_start(
            wv[:].rearrange("p (c f) -> p c f", c=8),
            wv_d.ap().rearrange("(c p) f -> p c f", p=128),
        )
        nc.gpsimd.dma_start(
            wq[:].rearrange("p (c f) -> p c f", c=8),
            wq_d.ap().rearrange("(c p) f -> p c f", p=128),
        )
        # wo packed 2 heads deep: partition (h%2)*64+d, free (h//2)*E+e
        nc.gpsimd.dma_start(
            wo2[:].rearrange("p (hh e) -> p hh e", hh=2),
            wo_d.ap().rearrange("(hh two d) e -> (two d) hh e", hh=2, two=2),
        )
        nc.gpsimd.dma_start(sbc[:], sbc_d.ap())
        # static contract rows 64..95 of qT/kT, replicated per head fold
        nc.gpsimd.dma_start(
            qT[64:96, :].rearrange("p (h s) -> p h s", h=HPC),
            qhot_d.ap().rearrange("p s -> p () s").broadcast_to((32, HPC, S)),
        )
        nc.gpsimd.dma_start(
            kT[64:96, :].rearrange("p (h s) -> p h s", h=HPC),
            kband_d.ap().rearrange("p s -> p () s").broadcast_to((32, HPC, S)),
        )

        # ---- phase 1: k projection (kT layout [head, f, s]) ----
        with tc.tile_pool(name="xk", bufs=4) as xkp, tc.tile_pool(
            name="psK", bufs=1, space="PSUM"
        ) as pskp:
            # phase-2 x preload: per-sc grouped loads issued near the end of
            # the xk stream — sc0 lands as phase-1 compute drains, later
            # chunks pipeline behind phase-2's sc-major consumption
            def xv_load(sc):
                nc.sync.dma_start(
                    xvbig[:, sc * 4096 : (sc + 1) * 4096].rearrange(
                        "p (c s) -> p c s", c=8
                    ),
                    xv_d.ap().rearrange("(c p) s -> p c s", p=128)[
                        :, :, sc * 512 : (sc + 1) * 512
                    ],
                )

            psK = pskp.tile([128, 4096], F32)
            for e in range(8):
                xt = xkp.tile([128, S], BF16, tag="xk")
                nc.sync.dma_start(xt[:], xk_d.ap()[e * 128 : (e + 1) * 128, :])
                if e == 7:
                    # after the last xk chunk: keep the xk stream unimpeded
                    for sc in range(4):
                        xv_load(sc)
                for fold in range(2):
                    for sc in range(4):
                        nc.tensor.matmul(
                            psK[:, (fold * 4 + sc) * 512 : (fold * 4 + sc + 1) * 512],
                            wk[:, e * F + fold * 128 : e * F + fold * 128 + 128],
                            xt[:, sc * 512 : (sc + 1) * 512],
                            start=(e == 0),
                            stop=(e == 7),
                        )
            for fold in range(2):
                for sc in range(4):
                    src = psK[:, (fold * 4 + sc) * 512 : (fold * 4 + sc + 1) * 512]
                    h0, h1 = 2 * fold, 2 * fold + 1
                    # split across scalar and DVE: vproj's PSUM reuse waits on
                    # this whole chain, so halve its length
                    nc.scalar.copy(
                        kT[0:64, h0 * S + sc * 512 : h0 * S + (sc + 1) * 512],
                        src[0:64, :],
                    )
                    nc.vector.tensor_copy(
                        kT[0:64, h1 * S + sc * 512 : h1 * S + (sc + 1) * 512],
                        src[64:128, :],
                    )

        # ---- phase 2: v projection (natural layout [s, f]) ----
        with tc.tile_pool(name="psV", bufs=2, space="PSUM") as psvp:
            for sc in range(4):
                pvs = [
                    psvp.tile([128, 256], F32, name=f"pv{sub}", tag=f"psV{sub}")
                    for sub in range(4)
                ]
                for e in range(8):
                    for sub in range(4):
                        nc.tensor.matmul(
                            pvs[sub][:],
                            xvbig[:, sc * 4096 + e * 512 + sub * 128 :
                                  sc * 4096 + e * 512 + (sub + 1) * 128],
                            wv[:, e * F : (e + 1) * F],
                            start=(e == 0),
                            stop=(e == 7),
                        )
                for sub in range(4):
                    # split PSUM->SBUF copies across scalar and DVE so the
                    # bank-recycle chain drains twice as fast
                    dst = vv[:, sc * 1024 + sub * 256 : sc * 1024 + (sub + 1) * 256]
                    if sub < 2:
                        nc.scalar.copy(dst, pvs[sub][:])
                    else:
                        nc.vector.tensor_copy(dst, pvs[sub][:])

        # ---- phase 3: q projection + attention + output projection ----
        xqp = ctx.enter_context(tc.tile_pool(name="xq", bufs=2))
        psSp = ctx.enter_context(tc.tile_pool(name="psS", bufs=6, space="PSUM"))
        flexp = ctx.enter_context(tc.tile_pool(name="flex", bufs=2, space="PSUM"))
        expp = ctx.enter_context(tc.tile_pool(name="expS", bufs=2))
        ptcp = ctx.enter_context(tc.tile_pool(name="ptc", bufs=2))
        rbsp = ctx.enter_context(tc.tile_pool(name="rbs", bufs=4))
        attp = ctx.enter_context(tc.tile_pool(name="att", bufs=4))
        outp = ctx.enter_context(tc.tile_pool(name="outsb", bufs=2))

        xq_tiles = {}

        def qproj_load(t):
            # one grouped load for the whole slab's x columns
            xt = xqp.tile([128, 8 * 512], BF16, tag="xq")
            nc.sync.dma_start(
                xt[:].rearrange("p (c s) -> p c s", c=8),
                xq_d.ap().rearrange("(c p) s -> p c s", p=128)[
                    :, :, t * 512 : (t + 1) * 512
                ],
            )
            xq_tiles[t] = xt

        def qproj_mm(t):
            xt = xq_tiles.pop(t)
            pqs = [
                flexp.tile([128, 512], F32, name=f"pq{fold}", tag="flex")
                for fold in range(2)
            ]
            for e in range(8):
                for fold in range(2):
                    nc.tensor.matmul(
                        pqs[fold][:],
                        wq[:, e * F + fold * 128 : e * F + fold * 128 + 128],
                        xt[:, e * 512 : (e + 1) * 512],
                        start=(e == 0),
                        stop=(e == 7),
                    )
            for fold in range(2):
                h0, h1 = 2 * fold, 2 * fold + 1
                nc.scalar.copy(
                    qT[0:64, h0 * S + t * QS : h0 * S + (t + 1) * QS],
                    pqs[fold][0:64, :],
                )
                nc.scalar.copy(
                    qT[0:64, h1 * S + t * QS : h1 * S + (t + 1) * QS],
                    pqs[fold][64:128, :],
                )

        def outproj(t, att2s):
            for sc2 in range(4):
                ob = outp.tile([128, 1024], BF16, tag="outsb")
                for eh in range(2):
                    po = flexp.tile([128, 512], F32, tag="flex")
                    for hh in range(2):
                        nc.tensor.matmul(
                            po[:],
                            att2s[hh][:, sc2 * 128 : sc2 * 128 + 128],
                            wo2[:, hh * E + eh * 512 : hh * E + eh * 512 + 512],
                            start=(hh == 0),
                            stop=(hh == 1),
                        )
                    nc.scalar.copy(ob[:, eh * 512 : (eh + 1) * 512], po[:])
                row = (4 * t + sc2) * 128
                nc.gpsimd.dma_start(out_d.ap()[row : row + 128, :], ob[:])

        # flat software pipeline over every (slab, head, pair) slot
        SLOTS = [
            (t, h, i)
            for t in range(T_SLABS)
            for h in range(HPC)
            for i in range(NP_T[t])
        ]
        N = len(SLOTS)
        ctx = {}
        atts_by_t = {t: [] for t in range(T_SLABS)}

        att2_by = {}

        def ensure_ctx(t, h):
            if (t, h) in ctx:
                return
            if h == 0 and t == 0:
                qproj_mm(0)
            if h % 2 == 0:
                att2_by[(t, h // 2)] = attp.tile(
                    [128, 512], BF16, name="att2", tag="att"
                )
            ctx[(t, h)] = {
                "expS": expp.tile([128, MAXP * QS], BF16, name="expS", tag="expS"),
                "ptc": ptcp.tile([128, MAXP * QS], BF16, name="ptc", tag="ptc"),
                "acco": psSp.tile([128, 512], F32, name="acco", tag="psS"),
                "attn": att2_by[(t, h // 2)],
            }
            if t + 1 < T_SLABS:
                if h == HPC - 2:
                    # start next slab's x transfer one unit before its matmuls
                    qproj_load(t + 1)
                elif h == HPC - 1:
                    qproj_mm(t + 1)

        def do_scores(g):
            t, h, i = SLOTS[g]
            ensure_ctx(t, h)
            c = ctx[(t, h)]
            j = PAIR_ORDER[t][i]
            c0 = LO[t] + 2 * j
            qlo, qhi = QRANGE[t][j]
            w = qhi - qlo
            ps = psSp.tile([128, 512], F32, name="ps", tag="psS")
            nc.tensor.matmul(
                ps[:, 0:w],
                kT[0:96, h * S + c0 * 64 : h * S + c0 * 64 + 128],
                qT[0:96, h * S + t * QS + qlo : h * S + t * QS + qhi],
                start=True,
                stop=True,
            )
            off = CUMOFF[t][i]
            nc.scalar.activation(c["expS"][:, off : off + w], ps[:, 0:w], EXP)

        def do_sums(g):
            t, h, i = SLOTS[g]
            c = ctx[(t, h)]
            # emit the full 512-wide chunks of the compact layout that pair
            # i's exp completed — sum/recip/normalize are block-agnostic
            for cidx in CHUNKS_AT[t].get(i, []):
                c0c = cidx * QS
                bs = psSp.tile([128, 512], F32, name="bs", tag="psS")
                nc.tensor.matmul(
                    bs[:],
                    sbc[:, :],
                    c["expS"][:, c0c : c0c + QS],
                    start=True,
                    stop=True,
                )
                rbs = rbsp.tile([128, 512], F32, tag="rbs")
                nc.vector.reciprocal_approx_fast(rbs[:], bs[:])
                # alternate the normalize multiply between DVE and the idle
                # Pool engine; keep unit-tail chunks on the faster DVE
                eng = (
                    nc.gpsimd
                    if (cidx % 2 == 1 and cidx < NCHUNK[t] - 2)
                    else nc.vector
                )
                eng.tensor_mul(
                    c["ptc"][:, c0c : c0c + QS], c["expS"][:, c0c : c0c + QS],
                    rbs[:],
                )

        def do_av(g):
            t, h, i = SLOTS[g]
            c = ctx[(t, h)]
            j = PAIR_ORDER[t][i]
            c0 = LO[t] + 2 * j
            qlo, qhi = QRANGE[t][j]
            w = qhi - qlo
            cp = c0 // 2
            off = CUMOFF[t][i]
            rb = 64 * (h % 2)  # odd heads land on PSUM partitions 64-127 so
            # the attn copy into the packed att2 tile stays partition-aligned
            nc.tensor.matmul(
                c["acco"][rb : rb + 64, qlo:qhi],
                vv[:, cp * F + h * 64 : cp * F + h * 64 + 64],
                c["ptc"][:, off : off + w],
                start=(i == 0),
                stop=(i == NP_T[t] - 1),
                skip_group_check=True,
            )
            if i == NP_T[t] - 1:
                nc.scalar.copy(
                    c["attn"][rb : rb + 64, :], c["acco"][rb : rb + 64, :]
                )
                if h % 2 == 1:
                    atts_by_t[t].append(c["attn"])
                    if len(atts_by_t[t]) == 2:
                        outproj(t, atts_by_t[t])

        qproj_load(0)  # transfer overlaps phase-2 compute
        do_scores(0)
        do_scores(1)
        for g in range(N):
            do_sums(g)
            if g + 2 < N:
                do_scores(g + 2)
            if g >= 3:
                do_av(g - 3)
        do_av(N - 3)
        do_av(N - 2)
        do_av(N - 1)

    nc.compile()
    return nc


_NC_CACHE = []


def _get_nc():
    if not _NC_CACHE:
        _NC_CACHE.append(build_nc())
    return _NC_CACHE[0]


def _host_consts():
    qhot = np.zeros((32, S), np.float32)
    for s in range(S):
        qhot[s // BLK, s] = 1.0
    kband = np.zeros((32, S), np.float32)
    for k in range(S):
        c = k // BLK
        for r in range(32):
            if abs(r - c) > BAND:
                kband[r, k] = BIGNEG
    sbc = np.full((128, 128), EPS_BG, np.float32)
    for k in range(128):
        for p in range(128):
            if k // 64 == p // 64:
                sbc[k, p] = 1.0
    return qhot, kband, sbc


def make_in_maps(query, key, value, Wq, Wk, Wv, Wo):
    query = np.asarray(query, np.float32)
    key = np.asarray(key, np.float32)
    value = np.asarray(value, np.float32)
    Wq = np.asarray(Wq, np.float32)
    Wk = np.asarray(Wk, np.float32)
    Wv = np.asarray(Wv, np.float32)
    Wo = np.asarray(Wo, np.float32)

    qhot, kband, sbc = _host_consts()

    in_maps = []
    for c in range(NCORES):
        b, g = divmod(c, HPC)
        fs = slice(F * g, F * (g + 1))
        in_maps.append(
            {
                "xqT": np.ascontiguousarray(query[b].T).astype(BF16NP),
                "xkT": np.ascontiguousarray(key[b].T).astype(BF16NP),
                "xvT": np.ascontiguousarray(value[b].T).astype(BF16NP),
                "wqT": np.ascontiguousarray((Wq[fs, :] * SCALE).T).astype(BF16NP),
                "wkT": np.ascontiguousarray(Wk[fs, :].T).astype(BF16NP),
                "wvT": np.ascontiguousarray(Wv[fs, :].T).astype(BF16NP),
                "woT": np.ascontiguousarray(Wo[:, fs].T).astype(BF16NP),
                "qhot": qhot.astype(BF16NP),
                "kband": kband.astype(BF16NP),
                "sbc": sbc.astype(BF16NP),
            }
        )
    return in_maps


def kernel(query, key, value, Wq, Wk, Wv, Wo):
    nc = _get_nc()
    in_maps = make_in_maps(query, key, value, Wq, Wk, Wv, Wo)
    res = bass_utils.run_bass_kernel_spmd(nc, in_maps, core_ids=list(range(NCORES)))
    out = np.zeros((B, S, E), np.float32)
    for c in range(NCORES):
        b = c // HPC
        out[b] += res.results[c]["out"]
    return out
